# revision 45
# baseline (speedup 1.0000x reference)
"""Trainium2 Bass kernel for MultiHeadSelfAttention (nn_MultiHeadSelfAttentionKVCache).

Reference computation (bs=2, seq=2048, dim=1024, H=16 heads, dh=64):
  q/k/v = x @ W.T + b            (per-head slices)
  attn  = softmax(where(mask==0, -1e-9, q k^T / 8))
  out   = attn @ v               -> (b, h, s, dh)
  out   = out.swapaxes(-1,-2).reshape(bs, seq, dim)   (faithful layout quirk)
  y     = out @ Wo.T + bo

Sharding: core c = b*4+g handles batch b, heads 4g..4g+3. The reshape quirk
makes final output rows 128*h..128*h+127 depend only on head h, so every core
is fully independent (no collectives).

Per-core kernel (all matmul operands bf16, fp32 PSUM accumulate):
  - S^T blocks = K Q^T (k on partitions) so PV runs with V stationary; the two
    heads of a pair run as row-tiled matmuls (tile_position (0,0)/(64,0)) which
    execute concurrently on the PE.
  - exp on ScalarE; masked logits give exp(-1e-9)=1.0 exactly. Causality is
    exploited at 128-column granularity: diagonal-band k-tile t only computes
    q-columns >= 128*t; its 128x128 triangle is fixed up with copy_predicated;
    everything fully above the diagonal is replaced by per-128-column-group
    V-column suffix sums added during the psum->sbuf copy (broadcast AP).
  - V is augmented with a ones column: PV matmul row 64 accumulates the
    softmax denominator for free.
  - O^T (+suffix) is PE-transposed to q-partitions 4 tiles at a time into one
    psum tile; one reciprocal over the 4 denominators + one broadcast
    tensor_tensor multiply normalizes into obuf.
  - Output projection consumes O tiles through a strided AP that realizes the
    reference's swapaxes/reshape for free; bo is added during the psum->sbuf
    copy against a host-broadcast [128, D] bias tile.
  - Inputs are staged in SBUF layout host-side with coalesced dma_starts
    (each DMA_DIRECT2D costs ~700ns of serial Sync-engine issue); order:
    W(v), idb, x[qc0], W(k), W(q), x[qc1], smalls, x[qc2..3], Wo. A matmul
    warmup bridges the ~9us DMA lead-in and warms the PE HAM clock gate;
    a dummy exp pre-triggers the ~2.7us ACT table-set load. Both pairs'
    attention runs as ONE 80-visit stream (no inter-pair seam) with S^T of
    visit v+1 emitted before PV of visit v.
  - Emission is ONE software-pipelined stream from the first projection to
    the last y chunk: pair-0's attention chunks run during the x load right
    behind the projection groups they depend on (pair-0's suffix-adds are
    deferred — psum is plain-copied to SBUF immediately, the suffix and
    transposes applied once the V colsum lands); pair-1's projections and
    pair-0's y ride scheduled visit slots; pair-1's y is the tail. The Tile
    scheduler closely follows per-engine emission order, so emission order
    is arranged to match DMA arrival order.

Measured (8 cores, axon TRN2, healthy 2.4GHz clock): HW exec ~166-169 us,
rel L2 err 3.46e-3 (session-start baseline: 211 us). NOTE: the shared device
sometimes downclocks to ~2.0GHz (N=512 matmuls show ~454ns instead of ~379ns
in the trace) — run-to-run timings are only comparable at equal clock.
"""

import sys

if "/opt/trn_rl_repo" not in sys.path:
    sys.path.insert(0, "/opt/trn_rl_repo")

import ml_dtypes
import numpy as np

import concourse.bass as bass
import concourse.tile as tile
from concourse import bacc, mybir
from concourse.bass_utils import run_bass_kernel_spmd

BF = mybir.dt.bfloat16
F32 = mybir.dt.float32
U8 = mybir.dt.uint8
BFNP = ml_dtypes.bfloat16

P = 128
S = 2048
D = 1024
H = 16
DH = 64
NE = D // P      # 8 e-tiles
QC = 512         # q-chunk width
NQC = S // QC    # 4
NKT = S // P     # 16 k-tiles
NCORES = 8
SCALE = DH ** (-0.5)


def build_nc():
    nc = bacc.Bacc("TRN2", target_bir_lowering=False, debug=False,
                   num_devices=NCORES)

    xd = nc.dram_tensor("xd", [P, NE, S], BF, kind="ExternalInput").ap()
    wd = nc.dram_tensor("wd", [P, 6, NE, P], BF, kind="ExternalInput").ap()
    bqkv = nc.dram_tensor("bqkv", [P, 6], F32, kind="ExternalInput").ap()
    mtri = nc.dram_tensor("mtri", [P, P], U8, kind="ExternalInput").ap()
    wod = nc.dram_tensor("wod", [P, NE, D], BF, kind="ExternalInput").ap()
    bobd = nc.dram_tensor("bob", [P, D], BF, kind="ExternalInput").ap()
    cntd = nc.dram_tensor("cnt", [1, 17], F32, kind="ExternalInput").ap()
    idbd = nc.dram_tensor("idb", [P, P], BF, kind="ExternalInput").ap()
    y = nc.dram_tensor("y", [4 * P, D], F32, kind="ExternalOutput").ap()

    with tile.TileContext(nc) as tc:
        with (
            tc.tile_pool(name="persist", bufs=1) as persist,
            tc.tile_pool(name="vt", bufs=2) as vt_pool,
            tc.tile_pool(name="et", bufs=8) as et_pool,
            tc.tile_pool(name="osb", bufs=6) as osb_pool,
            tc.tile_pool(name="rc", bufs=4) as rc_pool,
            tc.tile_pool(name="ysb", bufs=3) as y_pool,
            tc.tile_pool(name="stp", bufs=2, space="PSUM") as st_psum,
            tc.tile_pool(name="otp", bufs=2, space="PSUM") as ot_psum,
            tc.tile_pool(name="msp", bufs=2, space="PSUM") as misc_psum,
        ):
            # ---------- persistent tiles ----------
            xsb = persist.tile([P, NE, S], BF)
            wsb = persist.tile([P, 6, NE, P], BF)
            bsb = persist.tile([P, 6], F32)
            mtsb = persist.tile([P, P], U8)
            wosb = persist.tile([P, NE, D], BF)
            bob = persist.tile([P, D], BF)
            idb = persist.tile([P, P], BF)
            qtk = persist.tile([P, 2, 2, S], BF)        # (pair, q/k, s)
            vbuf = persist.tile([P, 2, NKT, 130], BF)   # (pair, kt, VA|1|VB|1)
            colsum = persist.tile([P, 2, NKT], F32)
            sufq = persist.tile([P, 2, 17], F32)        # rev-window sums
            sufA = persist.tile([P, 2, 17], F32)        # rows 0:64 dh, 64 cnt
            sufB = persist.tile([P, 2, 17], F32)
            obuf = persist.tile([P, 4, NE, DH, 2], BF)  # (head, ct, dh, j)

            # ---------- DMA emission (issue order = priority) ----------
            # host lays wd out j-order (2,5,1,4,0,3) so V/k/q weights are
            # contiguous batches; one dma_start each keeps Sync issue short
            nc.sync.dma_start(bsb, bqkv)
            nc.sync.dma_start(wsb[:, 0:2], wd[:, 0:2])     # V weights
            nc.sync.dma_start(idb, idbd)           # needed by first vtrans
            nc.sync.dma_start(xsb[:, :, 0:QC], xd[:, :, 0:QC])
            nc.sync.dma_start(wsb[:, 2:4], wd[:, 2:4])     # k weights
            nc.sync.dma_start(wsb[:, 4:6], wd[:, 4:6])     # q weights
            nc.sync.dma_start(xsb[:, :, QC:2 * QC], xd[:, :, QC:2 * QC])
            nc.sync.dma_start(mtsb, mtri)
            nc.sync.dma_start(bob, bobd)
            for p in (0, 1):                       # masked-count rows
                nc.sync.dma_start(sufA[64:65, p, :], cntd)
                nc.sync.dma_start(sufB[64:65, p, :], cntd)
            for qc in range(2, NQC):               # remaining x slabs
                qs = slice(qc * QC, (qc + 1) * QC)
                nc.sync.dma_start(xsb[:, :, qs], xd[:, :, qs])
            nc.sync.dma_start(wosb, wod)           # Wo only needed late

            # ---------- memsets ----------
            ones_t = persist.tile([P, 1024], BF)
            nc.vector.memset(ones_t, 1.0)
            nc.vector.memset(vbuf[:, :, :, 64:65], 1.0)
            nc.vector.memset(vbuf[:, :, :, 129:130], 1.0)
            nc.vector.memset(sufq[:, :, 16:17], 0.0)
            nc.vector.memset(sufA[0:64, :, 16:17], 0.0)
            nc.vector.memset(sufB[0:64, :, 16:17], 0.0)

            # pre-trigger the ~2.7us exp table-set load while DMAs land
            scr = persist.tile([1, 8], BF)
            nc.scalar.activation(out=scr, in_=ones_t[0:1, 0:8],
                                 func=mybir.ActivationFunctionType.Exp,
                                 scale=SCALE)

            # HAM warmup: keep PE busy until the first x slab lands (~14us)
            warm = ot_psum.tile([P, QC], F32, tag="ot", name="warm")
            for _ in range(108):
                nc.tensor.matmul(warm[:, 0:P], ones_t[:, 0:P], ones_t[:, 0:P],
                                 start=True, stop=True)

            # ---------- chunk emitters (software-pipelined emission) ----
            vts0 = vt_pool.tile([P, S], BF, tag="vts")
            vts1 = vt_pool.tile([P, S], BF, tag="vts")
            vts_tiles = [vts0, vts1]

            # wd/wsb/bsb column order: (V p0, V p1, k p0, k p1, q p0, q p1)
            # so the V and k/q weight DMAs are single contiguous batches
            def wslot(p, wi):
                return {2: 0, 5: 1, 1: 2, 4: 3, 0: 4, 3: 5}[3 * p + wi]

            def proj_chunk(p, wi, qc):
                j = wslot(p, wi)
                ps = misc_psum.tile([P, QC], F32, tag="m")
                for e in range(NE):
                    nc.tensor.matmul(
                        ps, wsb[:, j, e, :], xsb[:, e, qc * QC:(qc + 1) * QC],
                        start=(e == 0), stop=(e == NE - 1))
                if wi < 2:
                    dst = qtk[:, p, wi, qc * QC:(qc + 1) * QC]
                else:
                    dst = vts_tiles[p][:, qc * QC:(qc + 1) * QC]
                # bias-copy on DVE for both pairs: pair-0's copies share the
                # head window with 24 visits' exps, so ACT must stay exp-only
                nc.vector.tensor_scalar_add(
                    out=dst, in0=ps, scalar1=bsb[:, j:j + 1])

            def colsum_suffix(p):
                vts = vts_tiles[p]
                nc.vector.tensor_reduce(
                    out=colsum[:, p, :],
                    in_=vts.rearrange("a (t k) -> a t k", k=P),
                    axis=mybir.AxisListType.X, op=mybir.AluOpType.add)
                for k0 in range(1, NKT):
                    nc.vector.tensor_reduce(
                        out=sufq[:, p, k0:k0 + 1],
                        in_=colsum[:, p, k0:NKT],
                        axis=mybir.AxisListType.X, op=mybir.AluOpType.add)
                nc.sync.dma_start(sufA[0:64, p, 0:16], sufq[0:64, p, 0:16])
                nc.sync.dma_start(sufB[0:64, p, 0:16], sufq[64:128, p, 0:16])

            def vtrans_chunk(p, kt0):
                vts = vts_tiles[p]
                for kt in (kt0, kt0 + 1):
                    trp = misc_psum.tile([P, QC], BF, tag="m")
                    nc.tensor.transpose(
                        trp[:, 0:P], vts[:, kt * P:(kt + 1) * P], idb)
                    dst = vbuf[:, p, kt, :].rearrange(
                        "a (h c) -> a h c", h=2)[:, :, 0:64]
                    src = trp[:, 0:P].rearrange("a (h c) -> a h c", h=2)
                    nc.vector.tensor_copy(out=dst, in_=src)

            ysb_map = {}

            def y_chunk(h, ec):
                if ec == 0:
                    ysb_map[h] = y_pool.tile([P, D], F32, tag="ysb",
                                             name=f"ysb_{h}")
                ysb = ysb_map[h]
                es = slice(ec * QC, (ec + 1) * QC)
                yp = misc_psum.tile([P, QC], F32, tag="m")
                for ct in range(NE):
                    nc.tensor.matmul(
                        yp, obuf[:, h, ct, :, :], wosb[:, ct, es],
                        start=(ct == 0), stop=(ct == NE - 1))
                nc.vector.tensor_tensor(out=ysb[:, es], in0=yp,
                                        in1=bob[:, es],
                                        op=mybir.AluOpType.add)
                nc.sync.dma_start(y[h * P:(h + 1) * P, es], ysb[:, es])

            def y_chunks(p):
                return [lambda h=h, ec=ec: y_chunk(h, ec)
                        for h in (2 * p, 2 * p + 1) for ec in range(2)]

            def run_attention(allv, extra_sched, tail_extra=()):
                # allv: [(p, c, kt, qlo, first, last)] visit stream;
                # extra_sched: {visit_number: [callables]} injected after
                # that visit's S^T/PV emission
                it = 0
                pending = []
                ots = {}

                def side_group(p, h, osb, c):
                    # 4 PE transposes into one psum tile, then a single
                    # reciprocal over the 4 denominators and one broadcast
                    # multiply into obuf (tq = 4c+tt -> ct = 4c%8+tt, same j)
                    ct0, j = (4 * c) % NE, (4 * c) // NE
                    # inner dim padded to 66 so each transpose's PSUM offset
                    # (tt*132B) stays 4-byte aligned
                    trp4 = misc_psum.tile([P, 4, 66], BF, tag="m",
                                          name=f"trp4_{p}_{c}_{h}")
                    for tt in range(4):
                        nc.tensor.transpose(
                            trp4[:, tt, 0:65],
                            osb[0:65, tt * P:(tt + 1) * P],
                            idb[0:65, 0:65])
                    rc4 = rc_pool.tile([P, 4], F32, tag="rc")
                    nc.vector.reciprocal(
                        rc4, trp4[:, :, 64:65].rearrange("a b c -> a (b c)"))
                    nc.vector.tensor_tensor(
                        out=obuf[:, h, ct0:ct0 + 4, :, j],
                        in0=trp4[:, :, 0:64],
                        in1=rc4[:, :, None].broadcast_to([P, 4, DH]),
                        op=mybir.AluOpType.mult)

                def emit_st(p, c, kt, qlo):
                    qbase = c * QC
                    ks = slice(kt * P, (kt + 1) * P)
                    st = st_psum.tile([P, 1024], F32, tag="st")
                    # S^T = K Q^T, both heads row-tiled (contraction=64)
                    nc.tensor.matmul(
                        st[:, qlo:QC],
                        qtk[0:64, p, 1, ks],
                        qtk[0:64, p, 0, qbase + qlo:qbase + QC],
                        start=True, stop=True, tile_position=(0, 0))
                    nc.tensor.matmul(
                        st[:, QC + qlo:1024],
                        qtk[64:128, p, 1, ks],
                        qtk[64:128, p, 0, qbase + qlo:qbase + QC],
                        start=True, stop=True, tile_position=(64, 0))
                    et = et_pool.tile([P, 1024], BF)
                    if qlo == 0:
                        nc.scalar.activation(
                            out=et, in_=st,
                            func=mybir.ActivationFunctionType.Exp,
                            scale=SCALE)
                    else:
                        # both heads' live columns in one strided-AP call:
                        # the 352-cycle ACT fixed cost is paid once
                        etv = et.rearrange("a (h w) -> a h w", w=QC)
                        stv = st.rearrange("a (h w) -> a h w", w=QC)
                        nc.scalar.activation(
                            out=etv[:, :, qlo:QC], in_=stv[:, :, qlo:QC],
                            func=mybir.ActivationFunctionType.Exp,
                            scale=SCALE)
                    if kt >= 4 * c:  # diagonal: 128x128 triangle -> 1.0
                        etv = et.rearrange("a (h w) -> a h w", w=QC)
                        nc.vector.copy_predicated(
                            out=etv[:, :, qlo:qlo + P],
                            mask=mtsb[:, None, :].broadcast_to([P, 2, P]),
                            data=ones_t[:, 0:2 * P].rearrange(
                                "a (h w) -> a h w", h=2))
                    return et

                def emit_pv(p, c, kt, qlo, first, last, et):
                    if first:
                        ots[p, c] = (ot_psum.tile([P, QC], F32, tag="ot",
                                                  name=f"ota_{p}_{c}"),
                                     ot_psum.tile([P, QC], F32, tag="ot",
                                                  name=f"otb_{p}_{c}"))
                    ota, otb = ots[p, c]
                    # O^T += Vaug^T E^T  (row 64 = denominator)
                    nc.tensor.matmul(
                        ota[0:65, qlo:QC], vbuf[:, p, kt, 0:65],
                        et[:, qlo:QC],
                        start=first, stop=last, skip_group_check=True)
                    nc.tensor.matmul(
                        otb[0:65, qlo:QC], vbuf[:, p, kt, 65:130],
                        et[:, QC + qlo:1024],
                        start=first, stop=last, skip_group_check=True)
                    if not last:
                        return
                    for side in range(2):
                        h = 2 * p + side
                        ot = ota if side == 0 else otb
                        suf = sufA if side == 0 else sufB
                        osb = osb_pool.tile([P, QC], BF, tag="osb",
                                            name=f"osb_{p}_{c}_{side}")

                        def suffix_add(p=p, c=c, osb=osb, suf=suf):
                            nc.vector.tensor_tensor(
                                out=osb[0:65, :].rearrange(
                                    "a (g w) -> a g w", g=4),
                                in0=osb[0:65, :].rearrange(
                                    "a (g w) -> a g w", g=4),
                                in1=suf[0:65, p, 4 * c + 1:4 * c + 5][
                                    :, :, None].broadcast_to([65, 4, P]),
                                op=mybir.AluOpType.add)

                        if p == 0:
                            # pair 0 runs during the x load, before its
                            # colsum/suffix exists: plain-copy now (frees the
                            # ot psum bank immediately), suffix-add + the
                            # transposes once the suffix has landed
                            nc.vector.tensor_copy(out=osb[0:65, :],
                                                  in_=ot[0:65, :])
                            pending.append(
                                (28 + 2 * c,
                                 lambda p=p, h=h, osb=osb, c=c, sa=suffix_add:
                                 (sa(), side_group(p, h, osb, c))))
                        else:
                            nc.vector.tensor_tensor(
                                out=osb[0:65, :].rearrange(
                                    "a (g w) -> a g w", g=4),
                                in0=ot[0:65, :].rearrange(
                                    "a (g w) -> a g w", g=4),
                                in1=suf[0:65, p, 4 * c + 1:4 * c + 5][
                                    :, :, None].broadcast_to([65, 4, P]),
                                op=mybir.AluOpType.add)
                            pending.append(
                                (0, lambda p=p, h=h, osb=osb, c=c:
                                 side_group(p, h, osb, c)))

                # software pipeline: S^T of visit v+1 issues before PV of
                # visit v so the PE never waits on the exp in between;
                # pendings carry a ready-visit gate so deferred pair-0
                # suffix work stays out of the DVE queue until its inputs
                # are (about to be) available
                for cb in extra_sched.get(0, ()):
                    cb()
                staged = None
                for p, c, kt, qlo, first, last in allv:
                    et = emit_st(p, c, kt, qlo)
                    if staged is not None:
                        emit_pv(*staged)
                    staged = (p, c, kt, qlo, first, last, et)
                    it += 1
                    if pending and pending[0][0] <= it:
                        pending.pop(0)[1]()
                    for cb in extra_sched.get(it, ()):
                        cb()
                emit_pv(*staged)
                # the ot pool is free once the last chunk's osb copies are
                # emitted: a short dependency-free matmul burst keeps the
                # HAM clock-gate warm through the tail's DVE-paced stretch
                tailwarm = ot_psum.tile([P, QC], F32, tag="ot",
                                        name="tailwarm")
                for _ in range(8):
                    nc.tensor.matmul(tailwarm, ones_t[:, 0:P],
                                     ones_t[:, 0:QC], start=True, stop=True)
                tx = list(tail_extra)
                while pending or tx:
                    for _ in range(2):
                        if pending:
                            pending.pop(0)[1]()
                    if tx:
                        tx.pop(0)()

            # ---------- pipelined emission ----------
            # EVERYTHING is one 80-visit stream. Pair-0 chunk c's visits run
            # during the x load, right behind the projections they need
            # (emission order = PE execution order, so each x-gated proj
            # chunk is preceded by enough runnable visit work to cover the
            # slab wait); pair-1 work and pair-0's y ride later visits.
            # visit numbering: p0 chunks start at 1, 5, 13, 25; p1 at 41,
            # 45, 53, 65.
            allv = []
            for p in range(2):
                for c in range(NQC):
                    visits = ([(kt, 0) for kt in range(4 * c)]
                              + [(4 * c + t, P * t) for t in range(4)])
                    for ki, (kt, qlo) in enumerate(visits):
                        allv.append((p, c, kt, qlo, ki == 0,
                                     ki == len(visits) - 1))

            def qc_group(qc):
                def emit():
                    proj_chunk(0, 2, qc)           # V
                    proj_chunk(0, 1, qc)           # k
                    proj_chunk(0, 0, qc)           # q
                    vtrans_chunk(0, 4 * qc)
                    vtrans_chunk(0, 4 * qc + 2)
                    if qc == NQC - 1:
                        colsum_suffix(0)
                return emit

            sched = {0: [qc_group(0)], 4: [qc_group(1)],
                     12: [qc_group(2)], 24: [qc_group(3)]}

            def add(v, cb):
                sched.setdefault(v, []).append(cb)

            for qc in range(NQC):                  # pair-1 V + vtrans
                base = [1, 5, 13, 25][qc] + 1
                add(base, lambda qc=qc: proj_chunk(1, 2, qc))
                add(base + 1, lambda qc=qc: vtrans_chunk(1, 4 * qc))
                add(base + 2, lambda qc=qc: vtrans_chunk(1, 4 * qc + 2))
            add(29, lambda: colsum_suffix(1))
            for i, (wi, qc) in enumerate(          # pair-1 k/q, JIT
                    [(1, 0), (0, 0), (1, 1), (0, 1),
                     (1, 2), (0, 2), (1, 3), (0, 3)]):
                add(31 + 2 * i, lambda wi=wi, qc=qc: proj_chunk(1, wi, qc))
            yc0 = y_chunks(0)
            for i, cb in enumerate(yc0):
                # last two ride the ACT-bound pair-1 chunk-3 stretch
                add(50 + 8 * i, cb)                # 50,58,66,74

            run_attention(allv, sched, tail_extra=y_chunks(1))

    nc.compile()
    return nc


_NC = None


def _get_nc():
    global _NC
    if _NC is None:
        _NC = build_nc()
    return _NC


def _prep_core_inputs(cid, x, Wq, bq, Wk, bk, Wv, bv, Wo):
    b, g = cid // 4, cid % 4
    r0 = 256 * g  # first W-row (= output feature) of this core's 4 heads

    wd = np.empty((P, 6, NE, P), dtype=BFNP)
    bqkv = np.empty((P, 6), dtype=np.float32)
    Ws = (Wq, Wk, Wv)
    bs = (bq, bk, bv)
    slot = {2: 0, 5: 1, 1: 2, 4: 3, 0: 4, 3: 5}  # keep in sync with wslot
    for p in range(2):
        for wi in range(3):
            j = slot[3 * p + wi]
            rows = slice(r0 + P * p, r0 + P * (p + 1))
            blockT = np.ascontiguousarray(Ws[wi][rows, :].T)  # [D, 128]
            wd[:, j] = blockT.reshape(NE, P, P).transpose(1, 0, 2)
            bqkv[:, j] = bs[wi][rows]

    xT = np.ascontiguousarray(x[b].T)  # [D, S]
    xd = xT.reshape(NE, P, S).transpose(1, 0, 2).astype(BFNP)
    woT = np.ascontiguousarray(Wo.T)   # [D, D]
    wod = woT.reshape(NE, P, D).transpose(1, 0, 2).astype(BFNP)

    return {"xd": xd, "wd": wd, "bqkv": bqkv, "wod": wod}


def kernel(**inputs):
    x = np.asarray(inputs["x"], dtype=np.float32)
    Wq = np.asarray(inputs["Wq"], dtype=np.float32)
    bq = np.asarray(inputs["bq"], dtype=np.float32)
    Wk = np.asarray(inputs["Wk"], dtype=np.float32)
    bk = np.asarray(inputs["bk"], dtype=np.float32)
    Wv = np.asarray(inputs["Wv"], dtype=np.float32)
    bv = np.asarray(inputs["bv"], dtype=np.float32)
    Wo = np.asarray(inputs["Wo"], dtype=np.float32)
    bo = np.asarray(inputs["bo"], dtype=np.float32)

    cnt = np.zeros((1, 17), dtype=np.float32)
    for k0 in range(1, 17):
        cnt[0, k0] = float(P * (NKT - k0))

    shared = {
        "mtri": np.tril(np.ones((P, P), dtype=np.uint8), -1),
        "bob": np.broadcast_to(bo.reshape(1, D).astype(BFNP), (P, D)).copy(),
        "cnt": cnt,
        "idb": np.eye(P, dtype=BFNP),
    }

    in_maps = []
    for cid in range(NCORES):
        m = _prep_core_inputs(cid, x, Wq, bq, Wk, bk, Wv, bv, Wo)
        m.update(shared)
        in_maps.append(m)

    nc = _get_nc()
    res = run_bass_kernel_spmd(nc, in_maps, core_ids=list(range(NCORES)))

    out = np.empty((2, S, D), dtype=np.float32)
    for cid in range(NCORES):
        b, g = cid // 4, cid % 4
        out[b, 512 * g:512 * (g + 1), :] = res.results[cid]["y"]
    return out


if __name__ == "__main__":
    rng = np.random.default_rng(0)
    ins = {
        "x": rng.standard_normal((2, S, D), dtype=np.float32),
        "masks": np.tril(np.ones((S, S), dtype=np.float32)),
        "Wq": rng.standard_normal((D, D), dtype=np.float32) * 0.02,
        "bq": rng.standard_normal(D, dtype=np.float32) * 0.02,
        "Wk": rng.standard_normal((D, D), dtype=np.float32) * 0.02,
        "bk": rng.standard_normal(D, dtype=np.float32) * 0.02,
        "Wv": rng.standard_normal((D, D), dtype=np.float32) * 0.02,
        "bv": rng.standard_normal(D, dtype=np.float32) * 0.02,
        "Wo": rng.standard_normal((D, D), dtype=np.float32) * 0.02,
        "bo": rng.standard_normal(D, dtype=np.float32) * 0.02,
    }
    out = kernel(**ins)
    print("kernel ran, output shape", out.shape, "mean", out.mean())


# revision 46
# speedup vs baseline: 1.0151x; 1.0151x over previous
"""Trainium2 Bass kernel for MultiHeadSelfAttention (nn_MultiHeadSelfAttentionKVCache).

Reference computation (bs=2, seq=2048, dim=1024, H=16 heads, dh=64):
  q/k/v = x @ W.T + b            (per-head slices)
  attn  = softmax(where(mask==0, -1e-9, q k^T / 8))
  out   = attn @ v               -> (b, h, s, dh)
  out   = out.swapaxes(-1,-2).reshape(bs, seq, dim)   (faithful layout quirk)
  y     = out @ Wo.T + bo

Sharding: core c = b*4+g handles batch b, heads 4g..4g+3. The reshape quirk
makes final output rows 128*h..128*h+127 depend only on head h, so every core
is fully independent (no collectives).

Per-core kernel (all matmul operands bf16, fp32 PSUM accumulate):
  - S^T blocks = K Q^T (k on partitions) so PV runs with V stationary; the two
    heads of a pair run as row-tiled matmuls (tile_position (0,0)/(64,0)) which
    execute concurrently on the PE.
  - exp on ScalarE; masked logits give exp(-1e-9)=1.0 exactly. Causality is
    exploited at 128-column granularity: diagonal-band k-tile t only computes
    q-columns >= 128*t; its 128x128 triangle is fixed up with copy_predicated;
    everything fully above the diagonal is replaced by per-128-column-group
    V-column suffix sums added during the psum->sbuf copy (broadcast AP).
  - V is augmented with a ones column: PV matmul row 64 accumulates the
    softmax denominator for free.
  - O^T (+suffix) is PE-transposed to q-partitions 4 tiles at a time into one
    psum tile; one reciprocal over the 4 denominators + one broadcast
    tensor_tensor multiply normalizes into obuf.
  - Output projection consumes O tiles through a strided AP that realizes the
    reference's swapaxes/reshape for free; bo is added during the psum->sbuf
    copy against a host-broadcast [128, D] bias tile.
  - Inputs are staged in SBUF layout host-side with coalesced dma_starts
    (each DMA_DIRECT2D costs ~700ns of serial Sync-engine issue); order:
    W(v), idb, x[qc0], W(k), W(q), x[qc1], smalls, x[qc2..3], Wo. A matmul
    warmup bridges the ~9us DMA lead-in and warms the PE HAM clock gate;
    a dummy exp pre-triggers the ~2.7us ACT table-set load. Both pairs'
    attention runs as ONE 80-visit stream (no inter-pair seam) with S^T of
    visit v+1 emitted before PV of visit v.
  - Emission is ONE software-pipelined stream from the first projection to
    the last y chunk: pair-0's attention chunks run during the x load right
    behind the projection groups they depend on (pair-0's suffix-adds are
    deferred — psum is plain-copied to SBUF immediately, the suffix and
    transposes applied once the V colsum lands); pair-1's projections and
    pair-0's y ride scheduled visit slots; pair-1's y is the tail. The Tile
    scheduler closely follows per-engine emission order, so emission order
    is arranged to match DMA arrival order.

Measured (8 cores, axon TRN2, healthy 2.4GHz clock): HW exec ~166-169 us,
rel L2 err 3.46e-3 (session-start baseline: 211 us). NOTE: the shared device
sometimes downclocks to ~2.0GHz (N=512 matmuls show ~454ns instead of ~379ns
in the trace) — run-to-run timings are only comparable at equal clock.
"""

import sys

if "/opt/trn_rl_repo" not in sys.path:
    sys.path.insert(0, "/opt/trn_rl_repo")

import ml_dtypes
import numpy as np

import concourse.bass as bass
import concourse.tile as tile
from concourse import bacc, mybir
from concourse.bass_utils import run_bass_kernel_spmd

BF = mybir.dt.bfloat16
F32 = mybir.dt.float32
U8 = mybir.dt.uint8
BFNP = ml_dtypes.bfloat16

P = 128
S = 2048
D = 1024
H = 16
DH = 64
NE = D // P      # 8 e-tiles
QC = 512         # q-chunk width
NQC = S // QC    # 4
NKT = S // P     # 16 k-tiles
NCORES = 8
SCALE = DH ** (-0.5)


def build_nc():
    nc = bacc.Bacc("TRN2", target_bir_lowering=False, debug=False,
                   num_devices=NCORES)

    xd = nc.dram_tensor("xd", [P, NE, S], BF, kind="ExternalInput").ap()
    wd = nc.dram_tensor("wd", [P, 6, NE, P], BF, kind="ExternalInput").ap()
    bqkv = nc.dram_tensor("bqkv", [P, 6], F32, kind="ExternalInput").ap()
    mtri = nc.dram_tensor("mtri", [P, P], U8, kind="ExternalInput").ap()
    wod = nc.dram_tensor("wod", [P, NE, D], BF, kind="ExternalInput").ap()
    bobd = nc.dram_tensor("bob", [P, D], BF, kind="ExternalInput").ap()
    cntd = nc.dram_tensor("cnt", [1, 17], F32, kind="ExternalInput").ap()
    idbd = nc.dram_tensor("idb", [P, P], BF, kind="ExternalInput").ap()
    y = nc.dram_tensor("y", [4 * P, D], BF, kind="ExternalOutput").ap()

    with tile.TileContext(nc) as tc:
        with (
            tc.tile_pool(name="persist", bufs=1) as persist,
            tc.tile_pool(name="vt", bufs=2) as vt_pool,
            tc.tile_pool(name="et", bufs=8) as et_pool,
            tc.tile_pool(name="osb", bufs=6) as osb_pool,
            tc.tile_pool(name="rc", bufs=4) as rc_pool,
            tc.tile_pool(name="ysb", bufs=3) as y_pool,
            tc.tile_pool(name="stp", bufs=2, space="PSUM") as st_psum,
            tc.tile_pool(name="otp", bufs=2, space="PSUM") as ot_psum,
            tc.tile_pool(name="msp", bufs=2, space="PSUM") as misc_psum,
        ):
            # ---------- persistent tiles ----------
            xsb = persist.tile([P, NE, S], BF)
            wsb = persist.tile([P, 6, NE, P], BF)
            bsb = persist.tile([P, 6], F32)
            mtsb = persist.tile([P, P], U8)
            wosb = persist.tile([P, NE, D], BF)
            bob = persist.tile([P, D], BF)
            idb = persist.tile([P, P], BF)
            qtk = persist.tile([P, 2, 2, S], BF)        # (pair, q/k, s)
            vbuf = persist.tile([P, 2, NKT, 130], BF)   # (pair, kt, VA|1|VB|1)
            colsum = persist.tile([P, 2, NKT], F32)
            sufq = persist.tile([P, 2, 17], F32)        # rev-window sums
            sufA = persist.tile([P, 2, 17], F32)        # rows 0:64 dh, 64 cnt
            sufB = persist.tile([P, 2, 17], F32)
            obuf = persist.tile([P, 4, NE, DH, 2], BF)  # (head, ct, dh, j)

            # ---------- DMA emission (issue order = priority) ----------
            # host lays wd out j-order (2,5,1,4,0,3) so V/k/q weights are
            # contiguous batches; one dma_start each keeps Sync issue short
            nc.sync.dma_start(bsb, bqkv)
            nc.sync.dma_start(wsb[:, 0:2], wd[:, 0:2])     # V weights
            nc.sync.dma_start(idb, idbd)           # needed by first vtrans
            nc.sync.dma_start(xsb[:, 0:4, 0:QC], xd[:, 0:4, 0:QC])
            nc.sync.dma_start(xsb[:, 4:8, 0:QC], xd[:, 4:8, 0:QC])
            nc.sync.dma_start(wsb[:, 2:4], wd[:, 2:4])     # k weights
            nc.sync.dma_start(wsb[:, 4:6], wd[:, 4:6])     # q weights
            nc.sync.dma_start(xsb[:, :, QC:2 * QC], xd[:, :, QC:2 * QC])
            nc.sync.dma_start(mtsb, mtri)
            nc.sync.dma_start(bob, bobd)
            for p in (0, 1):                       # masked-count rows
                nc.sync.dma_start(sufA[64:65, p, :], cntd)
                nc.sync.dma_start(sufB[64:65, p, :], cntd)
            for qc in range(2, NQC):               # remaining x slabs
                qs = slice(qc * QC, (qc + 1) * QC)
                nc.sync.dma_start(xsb[:, :, qs], xd[:, :, qs])
            nc.sync.dma_start(wosb, wod)           # Wo only needed late

            # ---------- memsets ----------
            ones_t = persist.tile([P, 1024], BF)
            nc.vector.memset(ones_t, 1.0)
            nc.vector.memset(vbuf[:, :, :, 64:65], 1.0)
            nc.vector.memset(vbuf[:, :, :, 129:130], 1.0)
            nc.vector.memset(sufq[:, :, 16:17], 0.0)
            nc.vector.memset(sufA[0:64, :, 16:17], 0.0)
            nc.vector.memset(sufB[0:64, :, 16:17], 0.0)

            # pre-trigger the ~2.7us exp table-set load while DMAs land
            scr = persist.tile([1, 8], BF)
            nc.scalar.activation(out=scr, in_=ones_t[0:1, 0:8],
                                 func=mybir.ActivationFunctionType.Exp,
                                 scale=SCALE)

            # HAM warmup: keep PE busy until the first x slab lands (~14us)
            warm = ot_psum.tile([P, QC], F32, tag="ot", name="warm")
            for _ in range(108):
                nc.tensor.matmul(warm[:, 0:P], ones_t[:, 0:P], ones_t[:, 0:P],
                                 start=True, stop=True)

            # ---------- chunk emitters (software-pipelined emission) ----
            vts0 = vt_pool.tile([P, S], BF, tag="vts")
            vts1 = vt_pool.tile([P, S], BF, tag="vts")
            vts_tiles = [vts0, vts1]

            # wd/wsb/bsb column order: (V p0, V p1, k p0, k p1, q p0, q p1)
            # so the V and k/q weight DMAs are single contiguous batches
            def wslot(p, wi):
                return {2: 0, 5: 1, 1: 2, 4: 3, 0: 4, 3: 5}[3 * p + wi]

            def proj_chunk(p, wi, qc):
                j = wslot(p, wi)
                ps = misc_psum.tile([P, QC], F32, tag="m")
                for e in range(NE):
                    nc.tensor.matmul(
                        ps, wsb[:, j, e, :], xsb[:, e, qc * QC:(qc + 1) * QC],
                        start=(e == 0), stop=(e == NE - 1))
                if wi < 2:
                    dst = qtk[:, p, wi, qc * QC:(qc + 1) * QC]
                else:
                    dst = vts_tiles[p][:, qc * QC:(qc + 1) * QC]
                # bias-copy on DVE for both pairs: pair-0's copies share the
                # head window with 24 visits' exps, so ACT must stay exp-only
                nc.vector.tensor_scalar_add(
                    out=dst, in0=ps, scalar1=bsb[:, j:j + 1])

            def colsum_suffix(p):
                vts = vts_tiles[p]
                nc.vector.tensor_reduce(
                    out=colsum[:, p, :],
                    in_=vts.rearrange("a (t k) -> a t k", k=P),
                    axis=mybir.AxisListType.X, op=mybir.AluOpType.add)
                for k0 in range(1, NKT):
                    nc.vector.tensor_reduce(
                        out=sufq[:, p, k0:k0 + 1],
                        in_=colsum[:, p, k0:NKT],
                        axis=mybir.AxisListType.X, op=mybir.AluOpType.add)
                nc.sync.dma_start(sufA[0:64, p, 0:16], sufq[0:64, p, 0:16])
                nc.sync.dma_start(sufB[0:64, p, 0:16], sufq[64:128, p, 0:16])

            def vtrans_chunk(p, kt0):
                vts = vts_tiles[p]
                for kt in (kt0, kt0 + 1):
                    trp = misc_psum.tile([P, QC], BF, tag="m")
                    nc.tensor.transpose(
                        trp[:, 0:P], vts[:, kt * P:(kt + 1) * P], idb)
                    dst = vbuf[:, p, kt, :].rearrange(
                        "a (h c) -> a h c", h=2)[:, :, 0:64]
                    src = trp[:, 0:P].rearrange("a (h c) -> a h c", h=2)
                    nc.vector.tensor_copy(out=dst, in_=src)

            ysb_map = {}

            def y_chunk(h, ec):
                if ec == 0:
                    ysb_map[h] = y_pool.tile([P, D], BF, tag="ysb",
                                             name=f"ysb_{h}")
                ysb = ysb_map[h]
                es = slice(ec * QC, (ec + 1) * QC)
                yp = misc_psum.tile([P, QC], F32, tag="m")
                for ct in range(NE):
                    nc.tensor.matmul(
                        yp, obuf[:, h, ct, :, :], wosb[:, ct, es],
                        start=(ct == 0), stop=(ct == NE - 1))
                nc.vector.tensor_tensor(out=ysb[:, es], in0=yp,
                                        in1=bob[:, es],
                                        op=mybir.AluOpType.add)
                nc.sync.dma_start(y[h * P:(h + 1) * P, es], ysb[:, es])

            def y_chunks(p):
                return [lambda h=h, ec=ec: y_chunk(h, ec)
                        for h in (2 * p, 2 * p + 1) for ec in range(2)]

            def run_attention(allv, extra_sched, tail_extra=()):
                # allv: [(p, c, kt, qlo, first, last)] visit stream;
                # extra_sched: {visit_number: [callables]} injected after
                # that visit's S^T/PV emission
                it = 0
                pending = []
                ots = {}

                def side_group(p, h, osb, c):
                    # 4 PE transposes into one psum tile, then a single
                    # reciprocal over the 4 denominators and one broadcast
                    # multiply into obuf (tq = 4c+tt -> ct = 4c%8+tt, same j)
                    ct0, j = (4 * c) % NE, (4 * c) // NE
                    # inner dim padded to 66 so each transpose's PSUM offset
                    # (tt*132B) stays 4-byte aligned
                    trp4 = misc_psum.tile([P, 4, 66], BF, tag="m",
                                          name=f"trp4_{p}_{c}_{h}")
                    for tt in range(4):
                        nc.tensor.transpose(
                            trp4[:, tt, 0:65],
                            osb[0:65, tt * P:(tt + 1) * P],
                            idb[0:65, 0:65])
                    rc4 = rc_pool.tile([P, 4], F32, tag="rc")
                    nc.vector.reciprocal(
                        rc4, trp4[:, :, 64:65].rearrange("a b c -> a (b c)"))
                    nc.vector.tensor_tensor(
                        out=obuf[:, h, ct0:ct0 + 4, :, j],
                        in0=trp4[:, :, 0:64],
                        in1=rc4[:, :, None].broadcast_to([P, 4, DH]),
                        op=mybir.AluOpType.mult)

                def emit_st(p, c, kt, qlo):
                    qbase = c * QC
                    ks = slice(kt * P, (kt + 1) * P)
                    st = st_psum.tile([P, 1024], F32, tag="st")
                    # S^T = K Q^T, both heads row-tiled (contraction=64)
                    nc.tensor.matmul(
                        st[:, qlo:QC],
                        qtk[0:64, p, 1, ks],
                        qtk[0:64, p, 0, qbase + qlo:qbase + QC],
                        start=True, stop=True, tile_position=(0, 0))
                    nc.tensor.matmul(
                        st[:, QC + qlo:1024],
                        qtk[64:128, p, 1, ks],
                        qtk[64:128, p, 0, qbase + qlo:qbase + QC],
                        start=True, stop=True, tile_position=(64, 0))
                    et = et_pool.tile([P, 1024], BF)
                    if qlo == 0:
                        nc.scalar.activation(
                            out=et, in_=st,
                            func=mybir.ActivationFunctionType.Exp,
                            scale=SCALE)
                    else:
                        # both heads' live columns in one strided-AP call:
                        # the 352-cycle ACT fixed cost is paid once
                        etv = et.rearrange("a (h w) -> a h w", w=QC)
                        stv = st.rearrange("a (h w) -> a h w", w=QC)
                        nc.scalar.activation(
                            out=etv[:, :, qlo:QC], in_=stv[:, :, qlo:QC],
                            func=mybir.ActivationFunctionType.Exp,
                            scale=SCALE)
                    if kt >= 4 * c:  # diagonal: 128x128 triangle -> 1.0
                        etv = et.rearrange("a (h w) -> a h w", w=QC)
                        nc.vector.copy_predicated(
                            out=etv[:, :, qlo:qlo + P],
                            mask=mtsb[:, None, :].broadcast_to([P, 2, P]),
                            data=ones_t[:, 0:2 * P].rearrange(
                                "a (h w) -> a h w", h=2))
                    return et

                def emit_pv(p, c, kt, qlo, first, last, et):
                    if first:
                        ots[p, c] = (ot_psum.tile([P, QC], F32, tag="ot",
                                                  name=f"ota_{p}_{c}"),
                                     ot_psum.tile([P, QC], F32, tag="ot",
                                                  name=f"otb_{p}_{c}"))
                    ota, otb = ots[p, c]
                    # O^T += Vaug^T E^T  (row 64 = denominator)
                    nc.tensor.matmul(
                        ota[0:65, qlo:QC], vbuf[:, p, kt, 0:65],
                        et[:, qlo:QC],
                        start=first, stop=last, skip_group_check=True)
                    nc.tensor.matmul(
                        otb[0:65, qlo:QC], vbuf[:, p, kt, 65:130],
                        et[:, QC + qlo:1024],
                        start=first, stop=last, skip_group_check=True)
                    if not last:
                        return
                    for side in range(2):
                        h = 2 * p + side
                        ot = ota if side == 0 else otb
                        suf = sufA if side == 0 else sufB
                        osb = osb_pool.tile([P, QC], BF, tag="osb",
                                            name=f"osb_{p}_{c}_{side}")

                        def suffix_add(p=p, c=c, osb=osb, suf=suf):
                            nc.vector.tensor_tensor(
                                out=osb[0:65, :].rearrange(
                                    "a (g w) -> a g w", g=4),
                                in0=osb[0:65, :].rearrange(
                                    "a (g w) -> a g w", g=4),
                                in1=suf[0:65, p, 4 * c + 1:4 * c + 5][
                                    :, :, None].broadcast_to([65, 4, P]),
                                op=mybir.AluOpType.add)

                        if p == 0:
                            # pair 0 runs during the x load, before its
                            # colsum/suffix exists: plain-copy now (frees the
                            # ot psum bank immediately), suffix-add + the
                            # transposes once the suffix has landed
                            nc.vector.tensor_copy(out=osb[0:65, :],
                                                  in_=ot[0:65, :])
                            pending.append(
                                (28 + 2 * c,
                                 lambda p=p, h=h, osb=osb, c=c, sa=suffix_add:
                                 (sa(), side_group(p, h, osb, c))))
                        else:
                            nc.vector.tensor_tensor(
                                out=osb[0:65, :].rearrange(
                                    "a (g w) -> a g w", g=4),
                                in0=ot[0:65, :].rearrange(
                                    "a (g w) -> a g w", g=4),
                                in1=suf[0:65, p, 4 * c + 1:4 * c + 5][
                                    :, :, None].broadcast_to([65, 4, P]),
                                op=mybir.AluOpType.add)
                            pending.append(
                                (0, lambda p=p, h=h, osb=osb, c=c:
                                 side_group(p, h, osb, c)))

                # software pipeline: S^T of visit v+1 issues before PV of
                # visit v so the PE never waits on the exp in between;
                # pendings carry a ready-visit gate so deferred pair-0
                # suffix work stays out of the DVE queue until its inputs
                # are (about to be) available
                for cb in extra_sched.get(0, ()):
                    cb()
                staged = None
                for p, c, kt, qlo, first, last in allv:
                    et = emit_st(p, c, kt, qlo)
                    if staged is not None:
                        emit_pv(*staged)
                    staged = (p, c, kt, qlo, first, last, et)
                    it += 1
                    if pending and pending[0][0] <= it:
                        pending.pop(0)[1]()
                    for cb in extra_sched.get(it, ()):
                        cb()
                emit_pv(*staged)
                # the ot pool is free once the last chunk's osb copies are
                # emitted: a short dependency-free matmul burst keeps the
                # HAM clock-gate warm through the tail's DVE-paced stretch
                tailwarm = ot_psum.tile([P, QC], F32, tag="ot",
                                        name="tailwarm")
                for _ in range(8):
                    nc.tensor.matmul(tailwarm, ones_t[:, 0:P],
                                     ones_t[:, 0:QC], start=True, stop=True)
                tx = list(tail_extra)
                while pending or tx:
                    for _ in range(2):
                        if pending:
                            pending.pop(0)[1]()
                    if tx:
                        tx.pop(0)()

            # ---------- pipelined emission ----------
            # EVERYTHING is one 80-visit stream. Pair-0 chunk c's visits run
            # during the x load, right behind the projections they need
            # (emission order = PE execution order, so each x-gated proj
            # chunk is preceded by enough runnable visit work to cover the
            # slab wait); pair-1 work and pair-0's y ride later visits.
            # visit numbering: p0 chunks start at 1, 5, 13, 25; p1 at 41,
            # 45, 53, 65.
            allv = []
            for p in range(2):
                for c in range(NQC):
                    visits = ([(kt, 0) for kt in range(4 * c)]
                              + [(4 * c + t, P * t) for t in range(4)])
                    for ki, (kt, qlo) in enumerate(visits):
                        allv.append((p, c, kt, qlo, ki == 0,
                                     ki == len(visits) - 1))

            def qc_group(qc):
                def emit():
                    proj_chunk(0, 2, qc)           # V
                    proj_chunk(0, 1, qc)           # k
                    proj_chunk(0, 0, qc)           # q
                    vtrans_chunk(0, 4 * qc)
                    vtrans_chunk(0, 4 * qc + 2)
                    if qc == NQC - 1:
                        colsum_suffix(0)
                return emit

            sched = {0: [qc_group(0)], 4: [qc_group(1)],
                     12: [qc_group(2)], 24: [qc_group(3)]}

            def add(v, cb):
                sched.setdefault(v, []).append(cb)

            for qc in range(NQC):                  # pair-1 V + vtrans
                base = [1, 5, 13, 25][qc] + 1
                add(base, lambda qc=qc: proj_chunk(1, 2, qc))
                add(base + 1, lambda qc=qc: vtrans_chunk(1, 4 * qc))
                add(base + 2, lambda qc=qc: vtrans_chunk(1, 4 * qc + 2))
            add(29, lambda: colsum_suffix(1))
            for i, (wi, qc) in enumerate(          # pair-1 k/q, JIT
                    [(1, 0), (0, 0), (1, 1), (0, 1),
                     (1, 2), (0, 2), (1, 3), (0, 3)]):
                add(31 + 2 * i, lambda wi=wi, qc=qc: proj_chunk(1, wi, qc))
            yc0 = y_chunks(0)
            for i, cb in enumerate(yc0):
                # last two ride the ACT-bound pair-1 chunk-3 stretch
                add(50 + 8 * i, cb)                # 50,58,66,74

            run_attention(allv, sched, tail_extra=y_chunks(1))

    nc.compile()
    return nc


_NC = None


def _get_nc():
    global _NC
    if _NC is None:
        _NC = build_nc()
    return _NC


def _prep_core_inputs(cid, x, Wq, bq, Wk, bk, Wv, bv, Wo):
    b, g = cid // 4, cid % 4
    r0 = 256 * g  # first W-row (= output feature) of this core's 4 heads

    wd = np.empty((P, 6, NE, P), dtype=BFNP)
    bqkv = np.empty((P, 6), dtype=np.float32)
    Ws = (Wq, Wk, Wv)
    bs = (bq, bk, bv)
    slot = {2: 0, 5: 1, 1: 2, 4: 3, 0: 4, 3: 5}  # keep in sync with wslot
    for p in range(2):
        for wi in range(3):
            j = slot[3 * p + wi]
            rows = slice(r0 + P * p, r0 + P * (p + 1))
            blockT = np.ascontiguousarray(Ws[wi][rows, :].T)  # [D, 128]
            wd[:, j] = blockT.reshape(NE, P, P).transpose(1, 0, 2)
            bqkv[:, j] = bs[wi][rows]

    xT = np.ascontiguousarray(x[b].T)  # [D, S]
    xd = xT.reshape(NE, P, S).transpose(1, 0, 2).astype(BFNP)
    woT = np.ascontiguousarray(Wo.T)   # [D, D]
    wod = woT.reshape(NE, P, D).transpose(1, 0, 2).astype(BFNP)

    return {"xd": xd, "wd": wd, "bqkv": bqkv, "wod": wod}


def kernel(**inputs):
    x = np.asarray(inputs["x"], dtype=np.float32)
    Wq = np.asarray(inputs["Wq"], dtype=np.float32)
    bq = np.asarray(inputs["bq"], dtype=np.float32)
    Wk = np.asarray(inputs["Wk"], dtype=np.float32)
    bk = np.asarray(inputs["bk"], dtype=np.float32)
    Wv = np.asarray(inputs["Wv"], dtype=np.float32)
    bv = np.asarray(inputs["bv"], dtype=np.float32)
    Wo = np.asarray(inputs["Wo"], dtype=np.float32)
    bo = np.asarray(inputs["bo"], dtype=np.float32)

    cnt = np.zeros((1, 17), dtype=np.float32)
    for k0 in range(1, 17):
        cnt[0, k0] = float(P * (NKT - k0))

    shared = {
        "mtri": np.tril(np.ones((P, P), dtype=np.uint8), -1),
        "bob": np.broadcast_to(bo.reshape(1, D).astype(BFNP), (P, D)).copy(),
        "cnt": cnt,
        "idb": np.eye(P, dtype=BFNP),
    }

    in_maps = []
    for cid in range(NCORES):
        m = _prep_core_inputs(cid, x, Wq, bq, Wk, bk, Wv, bv, Wo)
        m.update(shared)
        in_maps.append(m)

    nc = _get_nc()
    res = run_bass_kernel_spmd(nc, in_maps, core_ids=list(range(NCORES)))

    out = np.empty((2, S, D), dtype=np.float32)
    for cid in range(NCORES):
        b, g = cid // 4, cid % 4
        out[b, 512 * g:512 * (g + 1), :] = res.results[cid]["y"].astype(np.float32)
    return out


if __name__ == "__main__":
    rng = np.random.default_rng(0)
    ins = {
        "x": rng.standard_normal((2, S, D), dtype=np.float32),
        "masks": np.tril(np.ones((S, S), dtype=np.float32)),
        "Wq": rng.standard_normal((D, D), dtype=np.float32) * 0.02,
        "bq": rng.standard_normal(D, dtype=np.float32) * 0.02,
        "Wk": rng.standard_normal((D, D), dtype=np.float32) * 0.02,
        "bk": rng.standard_normal(D, dtype=np.float32) * 0.02,
        "Wv": rng.standard_normal((D, D), dtype=np.float32) * 0.02,
        "bv": rng.standard_normal(D, dtype=np.float32) * 0.02,
        "Wo": rng.standard_normal((D, D), dtype=np.float32) * 0.02,
        "bo": rng.standard_normal(D, dtype=np.float32) * 0.02,
    }
    out = kernel(**ins)
    print("kernel ran, output shape", out.shape, "mean", out.mean())
